# revision 1
# baseline (speedup 1.0000x reference)
"""Multi-head attention (RoPE-full-dmodel variant) on 8 TRN2 NeuronCores.

Sharding: core c = (batch c//4, head-group c%4 of 4 heads).
 - W_q/W_k/W_v split column-wise by head (each core projects its 256 channels)
 - W_o split row-wise; per-core partial outputs summed on host (all-reduce at gather)

Per-core kernel (matmul data fp16, accumulation/softmax stats f32):
  RoPE(q,k) on DVE/GpSimd -> Q^T/K^T/V^T projections (chan-major) -> V seq-major
  via PE transpose (with a ones-column so the PV matmul also produces softmax
  denominators) -> scoresT = K_h^T-stationary x Q_h^T (K=64) -> exp on ACT over
  qblock-pairs (scale=1/8 folded in; no max-subtraction: scores ~ N(0,1)) ->
  U^T = (V|1)-stationary x expS^T -> normalize via reciprocal row-sums ->
  output projection interleaved per qblock-pair (row-parallel partial).

Layout trick: activations are host-transposed to D-major with an even/odd row
permutation of the d_model axis so the interleaved-repeat RoPE tables collapse
to 512 distinct rows, partition-aligned in 128-chunks.
"""
import os
import sys
from contextlib import nullcontext

for _p in ("/opt/trn_rl_repo", "/root/.axon_site/_ro/trn_rl_repo"):
    if os.path.isdir(_p) and _p not in sys.path:
        sys.path.insert(0, _p)

import numpy as np

import concourse.bacc as bacc
import concourse.tile as tile
import concourse.mybir as mybir
from concourse.bass_utils import run_bass_kernel_spmd

B, S, D = 2, 2048, 1024
H_TOT, DK = 16, 64
N_CORES, GROUPS = 8, 4
CH = 256            # channels (heads*dk) per core
KC = D // 128       # 8 d-model chunks
QB = S // 512       # 4 query blocks
ST = S // 128       # 16 seq tiles
HPC = 4             # heads per core
BASE = 10000.0

MM = mybir.dt.float16
F32 = mybir.dt.float32
AF = mybir.ActivationFunctionType

# even/odd permutation of the d_model axis: row r <- old d = 2r (r<512), 2(r-512)+1
_PERM = np.concatenate([np.arange(0, D, 2), np.arange(1, D, 2)])

_PROG = None


def _build(loop_n=1, phases=("v", "rope", "proj", "attn", "ut", "out")):
    nc = bacc.Bacc("TRN2", target_bir_lowering=False, debug=False)
    qT = nc.dram_tensor("qT", (D, S), MM, kind="ExternalInput").ap()
    kT = nc.dram_tensor("kT", (D, S), MM, kind="ExternalInput").ap()
    vT = nc.dram_tensor("vT", (D, S), MM, kind="ExternalInput").ap()
    cosc = nc.dram_tensor("cosc", (D // 2, S), MM, kind="ExternalInput").ap()
    sinc = nc.dram_tensor("sinc", (D // 2, S), MM, kind="ExternalInput").ap()
    wq = nc.dram_tensor("wq", (D, CH), MM, kind="ExternalInput").ap()
    wk = nc.dram_tensor("wk", (D, CH), MM, kind="ExternalInput").ap()
    wv = nc.dram_tensor("wv", (D, CH), MM, kind="ExternalInput").ap()
    wo = nc.dram_tensor("wo", (CH, D), MM, kind="ExternalInput").ap()
    bq = nc.dram_tensor("bq", (2, 128, 1), F32, kind="ExternalInput").ap()
    bk = nc.dram_tensor("bk", (2, 128, 1), F32, kind="ExternalInput").ap()
    bv = nc.dram_tensor("bv", (HPC, 64, 1), F32, kind="ExternalInput").ap()
    ident = nc.dram_tensor("ident", (128, 128), MM, kind="ExternalInput").ap()
    out = nc.dram_tensor("out", (S, D), MM, kind="ExternalOutput").ap()

    with tile.TileContext(nc) as tc:
      with (
          tc.tile_pool(name="consts", bufs=1) as consts,
          tc.tile_pool(name="qkv", bufs=1) as qkv,
          tc.tile_pool(name="misc", bufs=3) as misc,
          tc.tile_pool(name="outst", bufs=4) as outst,
          tc.tile_pool(name="ps_gen", bufs=2, space="PSUM") as ps_gen,
          tc.tile_pool(name="ps_sc", bufs=2, space="PSUM") as ps_sc,
          tc.tile_pool(name="ps_ut", bufs=2, space="PSUM") as ps_ut,
      ):
        with (tc.For_i(0, loop_n, 1, hint_engines=tuple(mybir.ALL_ENGINES))
              if loop_n > 1 else nullcontext()):
            # ---- small constants ----
            wq_sb = consts.tile([128, KC * CH], MM, tag="wq")
            wk_sb = consts.tile([128, KC * CH], MM, tag="wk")
            wv_sb = consts.tile([128, KC * CH], MM, tag="wv")
            ident_sb = consts.tile([128, 128], MM, tag="ident")
            for c in range(KC):
                nc.sync.dma_start(wv_sb[:, CH * c:CH * (c + 1)], wv[128 * c:128 * (c + 1), :])
            nc.sync.dma_start(ident_sb[:], ident)
            wo_sb = consts.tile([128, 2 * D], MM, tag="wo")
            bq_sb, bk_sb, bv_sb = [], [], []
            for c in range(2):
                t_ = consts.tile([128, 1], F32, tag=f"bq{c}", name=f"bq{c}")
                nc.sync.dma_start(t_[:], bq[c])
                bq_sb.append(t_)
                t_ = consts.tile([128, 1], F32, tag=f"bk{c}", name=f"bk{c}")
                nc.sync.dma_start(t_[:], bk[c])
                bk_sb.append(t_)
            for h in range(HPC):
                t_ = consts.tile([64, 1], F32, tag=f"bv{h}", name=f"bv{h}")
                nc.sync.dma_start(t_[:], bv[h])
                bv_sb.append(t_)

            # V storage: per (tile t, head h) block of 65 cols: 64 values + ones col
            v_sb = qkv.tile([128, ST * HPC * 65], MM, tag="v")
            ones_cols = v_sb[:].rearrange("p (b c) -> p b c", c=65)[:, :, 64]
            nc.vector.memset(ones_cols, 1.0)

            qt_sb = [qkv.tile([128, S], MM, tag=f"qt{c}", name=f"qt{c}") for c in range(2)]
            kt_sb = [qkv.tile([128, S], MM, tag=f"kt{c}", name=f"kt{c}") for c in range(2)]
            ut_sb = [qkv.tile([128, S], MM, tag=f"ut{c}", name=f"ut{c}") for c in range(2)]

            # ================= phase 1: RoPE + projections =================
            with (
                tc.tile_pool(name="p1", bufs=1) as p1,
            ):
                ps_proj = ps_gen
                # ---- rope-k inputs + tables first: they gate RoPE, the
                #      phase-1 critical path ----
                rope_pre = {}
                for a in (0, 2, 1, 3):
                    xa = p1.tile([128, S], MM, tag="ri", name=f"xk{a}", bufs=4)
                    nc.sync.dma_start(xa[:], kT[128 * a:128 * (a + 1), :])
                    rope_pre[a] = xa
                cos_sb, sin_sb = [None] * 4, [None] * 4
                for t in (0, 2, 1, 3):
                    ct = p1.tile([128, S], MM, tag=f"cos{t}", name=f"cos{t}")
                    nc.sync.dma_start(ct[:], cosc[128 * t:128 * (t + 1), :])
                    cos_sb[t] = ct
                    st_ = p1.tile([128, S], MM, tag=f"sin{t}", name=f"sin{t}")
                    nc.sync.dma_start(st_[:], sinc[128 * t:128 * (t + 1), :])
                    sin_sb[t] = st_

                # ---- V^T projection (chan-major) + PE transpose to seq-major ----
                vt_cm = [p1.tile([128, S], MM, tag=f"vtc{c}", name=f"vtc{c}", bufs=1)
                         for c in range(2)]
                if "v" in phases:
                    vchunks = []
                    for d in range(KC):
                        vc = p1.tile([128, S], MM, tag="roped", name=f"vch{d}", bufs=8)
                        nc.sync.dma_start(vc[:], vT[128 * d:128 * (d + 1), :])
                        vchunks.append(vc)
                    for c in range(2):
                        for sp in range(0, QB, 2):
                            psums = [ps_proj.tile([128, 512], F32, tag="proj", name="psum")
                                     for _ in range(2)]
                            for d in range(KC):
                                lhsT = wv_sb[:, CH * d + 128 * c: CH * d + 128 * (c + 1)]
                                for i in range(2):
                                    s_ = sp + i
                                    nc.tensor.matmul(psums[i][:], lhsT,
                                                     vchunks[d][:, 512 * s_:512 * (s_ + 1)],
                                                     start=(d == 0), stop=(d == KC - 1))
                            for i in range(2):
                                s_ = sp + i
                                nc.scalar.copy(vt_cm[c][:, 512 * s_:512 * (s_ + 1)], psums[i][:])
                    for t in range(ST):
                        pv = ps_gen.tile([128, CH], MM, tag="proj")
                        for c in range(2):
                            nc.tensor.transpose(pv[:, 128 * c:128 * (c + 1)],
                                                vt_cm[c][:, 128 * t:128 * (t + 1)],
                                                ident_sb[:])
                        dst = _v_scatter_ap(v_sb, t)
                        nc.scalar.copy(dst, pv[:])

                # ---- RoPE + projections: K first, then Q (q-proj split so
                #      attention qbp0 can start after qblocks 0-1 project) ----
                SPL = 1664  # DVE fp16 2x (245G/s) : GPSIMD 0.42x (64G/s)

                def _ew(op, out, in0, in1):
                    getattr(nc.vector, op)(out[:, :SPL], in0[:, :SPL], in1[:, :SPL])
                    getattr(nc.gpsimd, op)(out[:, SPL:], in0[:, SPL:], in1[:, SPL:])

                def _rope(src_t, pre=None):
                    roped = [None] * KC
                    for a in (0, 1, 4, 5):
                        b_ = a + 2
                        if pre and a in pre:
                            xa = pre.pop(a)
                        else:
                            xa = p1.tile([128, S], MM, tag="ri", name="xa", bufs=4)
                            nc.sync.dma_start(xa[:], src_t[128 * a:128 * (a + 1), :])
                        if pre and b_ in pre:
                            xb = pre.pop(b_)
                        else:
                            xb = p1.tile([128, S], MM, tag="ri", name="xb", bufs=4)
                            nc.sync.dma_start(xb[:], src_t[128 * b_:128 * (b_ + 1), :])
                        ca, cb = a % 4, b_ % 4
                        t1 = p1.tile([128, S], MM, tag="tmp", name="t1", bufs=3)
                        _ew("tensor_mul", t1, xa, cos_sb[ca])
                        t2 = p1.tile([128, S], MM, tag="tmp", name="t2", bufs=3)
                        _ew("tensor_mul", t2, xb, sin_sb[ca])
                        ra = p1.tile([128, S], MM, tag="roped", name="ra", bufs=8)
                        _ew("tensor_sub", ra, t1, t2)
                        t3 = p1.tile([128, S], MM, tag="tmp", name="t3", bufs=3)
                        _ew("tensor_mul", t3, xb, cos_sb[cb])
                        t4 = p1.tile([128, S], MM, tag="tmp", name="t4", bufs=3)
                        _ew("tensor_mul", t4, xa, sin_sb[cb])
                        rb = p1.tile([128, S], MM, tag="roped", name="rb", bufs=8)
                        _ew("tensor_add", rb, t3, t4)
                        roped[a], roped[b_] = ra, rb
                    return roped

                def _proj(roped, w_sb, b_sb, dst_tiles, s_list):
                    s_list = list(s_list)
                    for c in range(2):
                        for sp in range(0, len(s_list), 2):
                            pair = s_list[sp:sp + 2]
                            psums = [ps_proj.tile([128, 512], F32, tag="proj", name="psum")
                                     for _ in pair]
                            for d in range(KC):
                                lhsT = w_sb[:, CH * d + 128 * c: CH * d + 128 * (c + 1)]
                                for i, s_ in enumerate(pair):
                                    nc.tensor.matmul(psums[i][:], lhsT,
                                                     roped[d][:, 512 * s_:512 * (s_ + 1)],
                                                     start=(d == 0), stop=(d == KC - 1))
                            for i, s_ in enumerate(pair):
                                nc.scalar.activation(
                                    dst_tiles[c][:, 512 * s_:512 * (s_ + 1)], psums[i][:],
                                    AF.Identity, bias=b_sb[c][:])

                if "rope" in phases:
                    roped_k = _rope(kT, rope_pre)
                    for c in range(KC):
                        nc.sync.dma_start(wk_sb[:, CH * c:CH * (c + 1)],
                                          wk[128 * c:128 * (c + 1), :])
                        nc.sync.dma_start(wq_sb[:, CH * c:CH * (c + 1)],
                                          wq[128 * c:128 * (c + 1), :])
                    if "proj" in phases:
                        _proj(roped_k, wk_sb, bk_sb, kt_sb, range(QB))
                    roped_q = _rope(qT)
                    if "proj" in phases:
                        _proj(roped_q, wq_sb, bq_sb, qt_sb, range(QB))

            # ================= phase 2: attention + output projection =================
                for c in range(2):
                    nc.sync.dma_start(wo_sb[:, D * c:D * (c + 1)], wo[128 * c:128 * (c + 1), :])
                with (
                    tc.tile_pool(name="expp", bufs=10) as expp,
                ):
                    def _attention(qbp):
                        q_lo = 1024 * qbp
                        for h in range(HPC):
                            ct, po = h // 2, 64 * (h % 2)
                            qt_h = qt_sb[ct][po:po + 64, q_lo:q_lo + 1024]
                            kt_h = kt_sb[ct][po:po + 64, :]
                            puts = [ps_ut.tile([65, 512], F32, tag="ut", name=f"put{half}")
                                    for half in range(2)]
                            for t in range(ST):
                                psc = ps_sc.tile([128, 1024], F32, tag="sc", name="psc")
                                for half in range(2):
                                    nc.tensor.matmul(
                                        psc[:, 512 * half:512 * (half + 1)],
                                        kt_h[:, 128 * t:128 * (t + 1)],
                                        qt_h[:, 512 * half:512 * (half + 1)],
                                        start=True, stop=True)
                                e = expp.tile([128, 1024], MM, tag="e", name="e")
                                nc.scalar.activation(e[:], psc[:], AF.Exp, scale=0.125)
                                vs = v_sb[:, (t * HPC + h) * 65:(t * HPC + h) * 65 + 65]
                                if "ut" in phases:
                                    for half in range(2):
                                        nc.tensor.matmul(puts[half][:], vs,
                                                         e[:, 512 * half:512 * (half + 1)],
                                                         start=(t == 0), stop=(t == ST - 1),
                                                         skip_group_check=True)
                            for half in range(2 if "ut" in phases else 0):
                                qb = 2 * qbp + half
                                put = puts[half]
                                uraw = misc.tile([65, 512], F32, tag="uraw", name="uraw")
                                nc.vector.tensor_copy(uraw[:], put[:])
                                rec = misc.tile([1, 512], F32, tag="rec", name="rec")
                                nc.vector.reciprocal(rec[:], uraw[64:65, :])
                                bc = misc.tile([64, 512], F32, tag="bc", name="bc")
                                nc.gpsimd.partition_broadcast(bc[:], rec[:])
                                dst = ut_sb[ct][po:po + 64, 512 * qb:512 * (qb + 1)]
                                nc.vector.tensor_mul(dst, uraw[0:64, :], bc[:])
                                nc.vector.tensor_scalar_add(dst, dst, bv_sb[h][:])

                        if "out" in phases:
                            for st in range(8 * qbp, 8 * (qbp + 1)):
                                pos = [ps_gen.tile([128, 512], F32, tag="proj", name="po_")
                                       for _ in range(2)]
                                for cc in range(2):
                                    lhsT = ut_sb[cc][:, 128 * st:128 * (st + 1)]
                                    for nb in range(2):
                                        nc.tensor.matmul(
                                            pos[nb][:], lhsT,
                                            wo_sb[:, D * cc + 512 * nb: D * cc + 512 * (nb + 1)],
                                            start=(cc == 0), stop=(cc == 1))
                                for nb in range(2):
                                    stg = outst.tile([128, 512], MM, tag="stg", name="stg")
                                    nc.vector.tensor_copy(stg[:], pos[nb][:])
                                    nc.sync.dma_start(
                                        out[128 * st:128 * (st + 1), 512 * nb:512 * (nb + 1)],
                                        stg[:])

                    if "attn" in phases:
                        _attention(0)
                        _attention(1)
    nc.compile()
    return nc


def _v_scatter_ap(v_sb, t):
    """AP writing a [128, 256] chan-major block into the 65-strided V layout."""
    ap = v_sb[:, t * HPC * 65: t * HPC * 65 + HPC * 65]
    return ap.rearrange("p (h j) -> p h j", h=HPC)[:, :, 0:64]


def _prepare(q, k, v, Wq_w, Wq_b, Wk_w, Wk_b, Wv_w, Wv_b, Wo_w, Wo_b):
    f16 = np.float16
    pos = np.arange(1, S + 1, dtype=np.float32)
    theta = (BASE ** (-2.0 * np.arange(D // 2, dtype=np.float32) / D)).astype(np.float32)
    ang = theta[:, None] * pos[None, :]
    cosc = np.cos(ang).astype(f16)
    sinc = np.sin(ang).astype(f16)
    identity = np.eye(128, dtype=f16)

    per_batch = []
    for b in range(B):
        per_batch.append((
            np.ascontiguousarray(q[b].T[_PERM]).astype(f16),
            np.ascontiguousarray(k[b].T[_PERM]).astype(f16),
            np.ascontiguousarray(v[b].T).astype(f16),
        ))
    in_maps = []
    for c in range(N_CORES):
        b, g = divmod(c, GROUPS)
        rows = slice(CH * g, CH * (g + 1))
        qTb, kTb, vTb = per_batch[b]
        in_maps.append({
            "qT": qTb, "kT": kTb, "vT": vTb, "cosc": cosc, "sinc": sinc,
            "ident": identity,
            "wq": np.ascontiguousarray(Wq_w[rows, :].T[_PERM]).astype(f16),
            "wk": np.ascontiguousarray(Wk_w[rows, :].T[_PERM]).astype(f16),
            "wv": np.ascontiguousarray(Wv_w[rows, :].T).astype(f16),
            "wo": np.ascontiguousarray(Wo_w[:, rows].T).astype(f16),
            "bq": Wq_b[rows].astype(np.float32).reshape(2, 128, 1),
            "bk": Wk_b[rows].astype(np.float32).reshape(2, 128, 1),
            "bv": Wv_b[rows].astype(np.float32).reshape(HPC, 64, 1),
        })
    return in_maps


def kernel(q, k, v, Wq_w, Wq_b, Wk_w, Wk_b, Wv_w, Wv_b, Wo_w, Wo_b):
    global _PROG
    args = [np.asarray(x, dtype=np.float32) for x in
            (q, k, v, Wq_w, Wq_b, Wk_w, Wk_b, Wv_w, Wv_b, Wo_w, Wo_b)]
    if _PROG is None:
        _PROG = _build()
    in_maps = _prepare(*args)
    res = run_bass_kernel_spmd(_PROG, in_maps, core_ids=list(range(N_CORES)))
    kernel.last_results = res
    Wo_b32 = args[10]
    out = np.empty((B, S, D), dtype=np.float32)
    for b in range(B):
        acc = res.results[GROUPS * b]["out"].astype(np.float32)
        for g in range(1, GROUPS):
            acc += res.results[GROUPS * b + g]["out"]
        out[b] = acc + Wo_b32
    return out

